# revision 12
# baseline (speedup 1.0000x reference)
"""Trainium2 Bass kernel for the CMGCA cross-attention module.

Computation per sample (C=256, H=W=64, N=4096, 4 heads of 64 channels):
  g     = sigmoid(MLP(concat(mean_A, mean_B)))                    [C]
  msgA  = attn(WqA@A, WkB@B, WvB@B)   (channel-channel attention)
  A2    = A + WpA @ (msgA * g)
  msgB  = attn(WqB@B, WkA@A, WvA@A)
  B2    = B + WpB @ (msgB * g)

Key restructuring: q/k only feed the logits, so
  logits_A = (WqA Xa)(WkB Xb)^T = WqA (Xa Xb^T) WkB^T = WqA G WkB^T
with G = Xa Xb^T a [C, C] Gram matrix (contraction over N=4096 done once),
logits_B = WqB G^T WkA^T.  This removes the two big q/k convolutions.
X^T needed for the Gram contraction is produced with PE transposes; an
extra ones-column on the Gram rhs yields channel sums (gate input) free.

Sharding: pure data parallel, batch 16 -> 2 samples on each of 8 cores.
"""

import numpy as np

import concourse.bass as bass
import concourse.mybir as mybir
import concourse.tile as tile

F32 = mybir.dt.float32
F32R = mybir.dt.float32r
AX = mybir.AxisListType.X
AF = mybir.ActivationFunctionType

N_CORES = 8
B_FULL = 16
SPC = B_FULL // N_CORES  # samples per core
C = 256
HW = 64
N = HW * HW  # 4096
HEADS = 4
CH = C // HEADS  # 64
NB = N // 128    # 32 transpose blocks
NT = N // 512    # 8 streaming tiles
NQ = 4           # quarters of the transpose/Gram pipeline
NBQ = NB // NQ   # 8 blocks per quarter
SCALE = float(CH) ** -0.5  # 0.125


def _r(ap):
    """View an fp32 access pattern as float32r (full-rate PE matmul)."""
    return ap.bitcast(F32R)


def _split_excess_waits(nc, max_waits=1):
    """The walrus build in this container only lowers one sync-wait per
    instruction; move excess waits onto same-engine NOPs placed just before
    the instruction (engines execute in order, so this is equivalent)."""
    nsplit = 0
    for fn in nc.m.functions:
        for blk in fn.blocks:
            out = []
            changed = False
            for inst in blk.instructions:
                si = inst.sync_info
                waits = list(si.on_wait) if si and si.on_wait else []
                if len(waits) > max_waits:
                    extra = waits[:-max_waits]
                    for ci in range(0, len(extra), max_waits):
                        nop = mybir.InstNoOp(name=f"{inst.name}-ws{ci}")
                        nop.engine = inst.engine
                        nop.sync_info = mybir.SyncInfo(
                            on_wait=extra[ci : ci + max_waits], on_update=[]
                        )
                        nc.register_instruction(nop)
                        out.append(nop)
                        nsplit += 1
                    si.on_wait = waits[-max_waits:]
                    changed = True
                out.append(inst)
            if changed:
                blk.instructions[:] = out
    return nsplit


def build_nc():
    nc = bass.Bass(trn_type="TRN2", target_bir_lowering=False, debug=False)

    A = nc.declare_dram_parameter("A", [SPC, C, N], F32, isOutput=False)
    Bi = nc.declare_dram_parameter("B", [SPC, C, N], F32, isOutput=False)
    # all weight matrices are passed pre-transposed to [c_in, c_out]
    WQAT = nc.declare_dram_parameter("WQAT", [C, C], F32, isOutput=False)
    WKAT = nc.declare_dram_parameter("WKAT", [C, C], F32, isOutput=False)
    WVAT = nc.declare_dram_parameter("WVAT", [C, C], F32, isOutput=False)
    WPAT = nc.declare_dram_parameter("WPAT", [C, C], F32, isOutput=False)
    WQBT = nc.declare_dram_parameter("WQBT", [C, C], F32, isOutput=False)
    WKBT = nc.declare_dram_parameter("WKBT", [C, C], F32, isOutput=False)
    WVBT = nc.declare_dram_parameter("WVBT", [C, C], F32, isOutput=False)
    WPBT = nc.declare_dram_parameter("WPBT", [C, C], F32, isOutput=False)
    WG1T = nc.declare_dram_parameter("WG1T", [2 * C, C // 4], F32, isOutput=False)
    WG2T = nc.declare_dram_parameter("WG2T", [C // 4, C], F32, isOutput=False)
    BG1 = nc.declare_dram_parameter("BG1", [C // 4, 1], F32, isOutput=False)
    BG2 = nc.declare_dram_parameter("BG2", [C, 1], F32, isOutput=False)
    IDT128 = nc.declare_dram_parameter("IDT128", [128, 128], F32, isOutput=False)
    IDT2 = nc.declare_dram_parameter("IDT2", [128, CH], F32, isOutput=False)
    ONESC = nc.declare_dram_parameter("ONESC", [128, NBQ, 2], F32, isOutput=False)
    ZER128 = nc.declare_dram_parameter("ZER128", [128, 128], F32, isOutput=False)
    OA = nc.declare_dram_parameter("OA", [SPC, C, N], F32, isOutput=True)
    OB = nc.declare_dram_parameter("OB", [SPC, C, N], F32, isOutput=True)

    with tile.TileContext(nc) as tc:
        with (
            tc.tile_pool(name="singles", bufs=1) as singles,
            tc.tile_pool(name="xpool", bufs=1) as xpool,
            tc.tile_pool(name="xtpool", bufs=2) as xtpool,
            tc.tile_pool(name="gsb", bufs=1) as gsbp,
            tc.tile_pool(name="n1sb", bufs=2) as n1p,
            tc.tile_pool(name="softmax", bufs=4) as smp,
            tc.tile_pool(name="ptp", bufs=1) as ptp,
            tc.tile_pool(name="stream", bufs=3) as stream,
        ):
            # ---- load weights / constants (once) ----
            def wload(dram, shape, pattern, nm, rnd=True, **kw):
                t = singles.tile(shape, F32, name=nm, tag=nm)
                dst = _r(t) if rnd else t
                src_ap = dram[:, :].rearrange(pattern, **kw)
                nc.sync.dma_start(out=dst, in_=_r(src_ap) if rnd else src_ap)
                return t

            wqat = wload(WQAT, [128, 2, C], "(k p) o -> p k o", "wqat", p=128)
            wkat = wload(WKAT, [128, 2, C], "(k p) o -> p k o", "wkat", p=128)
            wvat = wload(WVAT, [128, 2, C], "(k p) o -> p k o", "wvat", p=128)
            wpat = wload(WPAT, [128, 2, C], "(k p) o -> p k o", "wpat", p=128)
            wqbt = wload(WQBT, [128, 2, C], "(k p) o -> p k o", "wqbt", p=128)
            wkbt = wload(WKBT, [128, 2, C], "(k p) o -> p k o", "wkbt", p=128)
            wvbt = wload(WVBT, [128, 2, C], "(k p) o -> p k o", "wvbt", p=128)
            wpbt = wload(WPBT, [128, 2, C], "(k p) o -> p k o", "wpbt", p=128)
            wg1t = wload(WG1T, [128, 4, C // 4], "(k p) o -> p k o", "wg1t", rnd=False, p=128)
            wg2t = wload(WG2T, [C // 4, 2, 128], "p (j o) -> p j o", "wg2t", rnd=False, j=2)
            bg1 = singles.tile([C // 4, 1], F32)
            nc.sync.dma_start(out=bg1, in_=BG1[:, :])
            bg2 = singles.tile([128, 2], F32)
            nc.sync.dma_start(out=bg2, in_=BG2[:, :].rearrange("(j p) o -> p (j o)", p=128))
            idt128 = singles.tile([128, 128], F32)
            nc.sync.dma_start(out=_r(idt128), in_=_r(IDT128[:, :]))
            idt2 = singles.tile([128, CH], F32)
            nc.sync.dma_start(out=idt2, in_=IDT2[:, :])

            for s in range(SPC):
                # ---- load inputs ----
                xa = []
                xb = []
                for k in range(2):
                    t = xpool.tile([128, N], F32, tag=f"xa{k}")
                    nc.sync.dma_start(out=_r(t), in_=_r(A[s, k * 128 : (k + 1) * 128, :]))
                    xa.append(t)
                for k in range(2):
                    t = xpool.tile([128, N], F32, tag=f"xb{k}")
                    nc.sync.dma_start(out=_r(t), in_=_r(Bi[s, k * 128 : (k + 1) * 128, :]))
                    xb.append(t)

                # ---- phase 1: X transposes + Gram matrices ----
                # G_ab = Xa Xb^T (plus ones col -> channel sums of Xa)
                # G_ba = Xb Xa^T (plus ones col -> channel sums of Xb)
                g_ab = gsbp.tile([128, 2, C + 2], F32, tag="g_ab")
                g_ba = gsbp.tile([128, 2, C + 2], F32, tag="g_ba")
                with (
                    tc.tile_pool(name="ps_xt", bufs=2, space="PSUM") as pxt,
                    tc.tile_pool(name="ps_g", bufs=1, space="PSUM") as pg,
                ):
                    gps = {
                        (d, ib): pg.tile([128, C + 2], F32, tag=f"g{d}{ib}", name=f"gps_{d}{ib}")
                        for d in ("ab", "ba")
                        for ib in range(2)
                    }
                    for q in range(NQ):
                        xtq = {}
                        for tn, src in (("a", xa), ("b", xb)):
                            xt = xtpool.tile([128, NBQ, C + 2], F32, tag=f"xt{tn}")
                            nc.sync.dma_start(out=_r(xt[:, :, C : C + 2]), in_=_r(ONESC[:, :, :]))
                            for pair in range(NBQ // 2):
                                ps = pxt.tile([128, 512], F32R, tag="xtps")
                                for half in range(2):
                                    nb = q * NBQ + pair * 2 + half
                                    for k in range(2):
                                        nc.tensor.transpose(
                                            out=ps[:, half * 256 + k * 128 : half * 256 + (k + 1) * 128],
                                            in_=_r(src[k][:, nb * 128 : (nb + 1) * 128]),
                                            identity=_r(idt128),
                                        )
                                nc.vector.tensor_copy(
                                    out=_r(xt[:, pair * 2 : pair * 2 + 2, 0:C]),
                                    in_=ps[:].rearrange("p (a b) -> p a b", a=2),
                                )
                            xtq[tn] = xt
                        for dir_, lh, rh in (("ab", "a", "b"), ("ba", "b", "a")):
                            for ib in range(2):
                                for j in range(NBQ):
                                    nc.tensor.matmul(
                                        gps[(dir_, ib)][:],
                                        lhsT=_r(xtq[lh][:, j, ib * 128 : (ib + 1) * 128]),
                                        rhs=_r(xtq[rh][:, j, :]),
                                        start=(q == 0 and j == 0),
                                        stop=(q == NQ - 1 and j == NBQ - 1),
                                    )
                    for ib in range(2):
                        nc.vector.tensor_copy(out=_r(g_ab[:, ib, :]), in_=gps[("ab", ib)][:])
                        nc.vector.tensor_copy(out=_r(g_ba[:, ib, :]), in_=gps[("ba", ib)][:])

                # ---- phase 2: gate MLP + logits + softmax ----
                g_gate = smp.tile([128, 2], F32, tag="g_gate")
                pts = {}
                sgs = {}
                with (
                    tc.tile_pool(name="ps_l", bufs=1, space="PSUM") as pl,
                    tc.tile_pool(name="ps_n1", bufs=2, space="PSUM") as pn1,
                    tc.tile_pool(name="ps_gate", bufs=1, space="PSUM") as pgt,
                ):
                    # gate: h1 = relu(wg1 @ pooled + bg1); g = sigmoid(wg2 @ h1 + bg2)
                    h1ps = pgt.tile([C // 4, 1], F32, tag="gateps")
                    cols = [g_ab[:, 0, C : C + 1], g_ab[:, 1, C : C + 1],
                            g_ba[:, 0, C : C + 1], g_ba[:, 1, C : C + 1]]
                    for kc in range(4):
                        nc.tensor.matmul(
                            h1ps[:], lhsT=wg1t[:, kc, :], rhs=cols[kc],
                            start=(kc == 0), stop=(kc == 3),
                        )
                    h1 = smp.tile([C // 4, 1], F32, tag="h1")
                    nc.scalar.activation(out=h1, in_=h1ps[:], func=AF.Relu,
                                         bias=bg1[:], scale=1.0 / N)
                    for jb in range(2):
                        gps2 = pgt.tile([128, 1], F32, tag="gateps2")
                        nc.tensor.matmul(gps2[:], lhsT=wg2t[:, jb, :], rhs=h1[:],
                                         start=True, stop=True)
                        nc.scalar.activation(out=g_gate[:, jb : jb + 1], in_=gps2[:],
                                             func=AF.Sigmoid, bias=bg2[:, jb : jb + 1],
                                             scale=1.0)

                    for side, gmat, wk, wq in (
                        ("A", g_ba, wkbt, wqat),
                        ("B", g_ab, wkat, wqbt),
                    ):
                        # N1 = G @ Wk^T   (lhsT = G^T = other-direction gram)
                        n1 = n1p.tile([128, 2, C], F32, tag="n1")
                        for ib in range(2):
                            n1ps = pn1.tile([128, C], F32, tag="n1ps")
                            for jc in range(2):
                                nc.tensor.matmul(
                                    n1ps[:],
                                    lhsT=_r(gmat[:, jc, ib * 128 : (ib + 1) * 128]),
                                    rhs=_r(wk[:, jc, :]),
                                    start=(jc == 0), stop=(jc == 1),
                                )
                            nc.vector.tensor_copy(out=_r(n1[:, ib, :]), in_=n1ps[:])
                        # logits = Wq @ N1 ; then per-head softmax
                        for ob in range(2):
                            lps = pl.tile([128, C], F32, tag=f"L{ob}")
                            for cc in range(2):
                                nc.tensor.matmul(
                                    lps[:],
                                    lhsT=_r(wq[:, cc, ob * 128 : (ob + 1) * 128]),
                                    rhs=_r(n1[:, cc, :]),
                                    start=(cc == 0), stop=(cc == 1),
                                )
                            negm = smp.tile([128, 1], F32, tag="negm")
                            p_t = smp.tile([128, CH], F32, tag="p_t")
                            for hh in range(2):
                                h = 2 * ob + hh
                                rows = slice(hh * 64, hh * 64 + 64)
                                dwin = slice(h * 64, h * 64 + 64)
                                nc.vector.reduce_max(out=negm[rows, :], in_=lps[rows, dwin], axis=AX)
                            nc.vector.tensor_scalar_mul(negm[:], negm[:], -SCALE)
                            for hh in range(2):
                                h = 2 * ob + hh
                                rows = slice(hh * 64, hh * 64 + 64)
                                dwin = slice(h * 64, h * 64 + 64)
                                nc.scalar.activation(out=p_t[rows, :], in_=lps[rows, dwin],
                                                     func=AF.Exp, bias=negm[rows, :],
                                                     scale=SCALE)
                            zt = smp.tile([128, 1], F32, tag="zt")
                            nc.vector.reduce_sum(out=zt[:], in_=p_t[:], axis=AX)
                            rz = smp.tile([128, 1], F32, tag="rz")
                            nc.vector.reciprocal(out=rz[:], in_=zt[:])
                            sg = smp.tile([128, 1], F32, tag="sg")
                            nc.vector.tensor_mul(sg[:], rz[:], g_gate[:, ob : ob + 1])
                            nc.vector.tensor_scalar_mul(p_t[:], p_t[:], sg[:])
                            sgs[(side, ob)] = sg
                            pts[(side, ob)] = p_t

                # ---- phase 3: P transposes + V conv + AV + proj + residual ----
                with tc.tile_pool(name="ps_av", bufs=2, space="PSUM") as pav:
                    ptblocks = {}
                    for side in ("A", "B"):
                        for ob in range(2):
                            ptb = ptp.tile([128, 128], F32, tag=f"pt{side}{ob}")
                            nc.sync.dma_start(out=_r(ptb[:]), in_=_r(ZER128[:, :]))
                            p_t = pts[(side, ob)]
                            for hh in range(2):
                                rows = slice(hh * 64, hh * 64 + 64)
                                tps = pav.tile([64, 64], F32, tag="tps")
                                nc.tensor.transpose(out=tps[:], in_=p_t[rows, :],
                                                    identity=idt2[rows, :])
                                nc.vector.tensor_copy(out=_r(ptb[rows, hh * 64 : hh * 64 + 64]),
                                                      in_=tps[:])
                            ptblocks[(side, ob)] = ptb

                    for side, xo, xr, wv, wp, odr in (
                        ("A", xb, xa, wvbt, wpat, OA),
                        ("B", xa, xb, wvat, wpbt, OB),
                    ):
                        for nt in range(NT):
                            ntw = slice(nt * 512, (nt + 1) * 512)
                            mgs = []
                            for i in range(2):
                                vps = pav.tile([128, 512], F32, tag="vps")
                                for k in range(2):
                                    nc.tensor.matmul(
                                        vps[:],
                                        lhsT=_r(wv[:, k, i * 128 : (i + 1) * 128]),
                                        rhs=_r(xo[k][:, ntw]),
                                        start=(k == 0), stop=(k == 1),
                                    )
                                vsb = stream.tile([128, 512], F32, tag="vsb")
                                if (nt + i) % 2 == 0:
                                    nc.scalar.copy(out=_r(vsb[:]), in_=vps[:])
                                else:
                                    nc.vector.tensor_copy(out=_r(vsb[:]), in_=vps[:])
                                avps = pav.tile([128, 512], F32, tag="avps")
                                nc.tensor.matmul(avps[:], lhsT=_r(ptblocks[(side, i)][:]),
                                                 rhs=_r(vsb[:]), start=True, stop=True)
                                mg = stream.tile([128, 512], F32, tag="mg")
                                if (nt + i) % 2 == 0:
                                    nc.vector.tensor_copy(out=_r(mg[:]), in_=avps[:])
                                else:
                                    nc.scalar.copy(out=_r(mg[:]), in_=avps[:])
                                mgs.append(mg)
                            for j in range(2):
                                prps = pav.tile([128, 512], F32, tag="prps")
                                for i in range(2):
                                    nc.tensor.matmul(
                                        prps[:],
                                        lhsT=_r(wp[:, i, j * 128 : (j + 1) * 128]),
                                        rhs=_r(mgs[i][:]),
                                        start=(i == 0), stop=(i == 1),
                                    )
                                osb = stream.tile([128, 512], F32, tag="osb")
                                nc.vector.tensor_add(osb[:], prps[:], xr[j][:, ntw])
                                nc.sync.dma_start(out=odr[s, j * 128 : (j + 1) * 128, ntw],
                                                  in_=osb[:])

    _split_excess_waits(nc)
    return nc


_NC_CACHE = None


def _get_nc():
    global _NC_CACHE
    if _NC_CACHE is None:
        _NC_CACHE = build_nc()
    return _NC_CACHE


def make_in_maps(inputs):
    """Host-side prep: shard batch across cores, pre-transpose weights."""
    f = lambda x: np.ascontiguousarray(np.asarray(x, dtype=np.float32))
    A = f(inputs["A"]).reshape(B_FULL, C, N)
    B = f(inputs["B"]).reshape(B_FULL, C, N)
    shared = {
        "WQAT": f(inputs["wqA"].T),
        "WKAT": f(inputs["wkA"].T),
        "WVAT": f(inputs["wvA"].T),
        "WPAT": f(inputs["wprojA"].T),
        "WQBT": f(inputs["wqB"].T),
        "WKBT": f(inputs["wkB"].T),
        "WVBT": f(inputs["wvB"].T),
        "WPBT": f(inputs["wprojB"].T),
        "WG1T": f(inputs["wg1"].T),
        "WG2T": f(inputs["wg2"].T),
        "BG1": f(inputs["bg1"]).reshape(C // 4, 1),
        "BG2": f(inputs["bg2"]).reshape(C, 1),
        "IDT128": np.eye(128, dtype=np.float32),
        "IDT2": np.concatenate([np.eye(CH, dtype=np.float32)] * 2, axis=0),
        "ONESC": np.ones((128, NBQ, 2), dtype=np.float32),
        "ZER128": np.zeros((128, 128), dtype=np.float32),
    }
    in_maps = []
    for c in range(N_CORES):
        m = dict(shared)
        m["A"] = A[c * SPC : (c + 1) * SPC]
        m["B"] = B[c * SPC : (c + 1) * SPC]
        in_maps.append(m)
    return in_maps


def kernel(**inputs):
    from concourse.bass_utils import run_bass_kernel_spmd

    nc = _get_nc()
    in_maps = make_in_maps(inputs)
    res = run_bass_kernel_spmd(nc, in_maps, list(range(N_CORES)))
    outs = res.results
    A2 = np.concatenate([outs[c]["OA"] for c in range(N_CORES)], axis=0)
    B2 = np.concatenate([outs[c]["OB"] for c in range(N_CORES)], axis=0)
    A2 = A2.reshape(B_FULL, C, HW, HW)
    B2 = B2.reshape(B_FULL, C, HW, HW)
    return (A2, B2)
